# revision 14
# baseline (speedup 1.0000x reference)
"""Trainium2 Bass kernel for nn_ActorCritic_38886633898257.

Computes, for each batch row b of x (B, S, E):
  pairs[t]  = concat(x[b, t], x[b, t+1])            t in [0, S-2]
  h         = relu(pairs @ W1 + b1)
  scores[t] = h @ W2[:, 0]                          (+ b2, shift-invariant)
  logits    = scores masked to t < len_b - 1
  logp      = log_softmax(logits)
  out[b]    = (logp[action_b], entropy(logits))

Strategy: pure data parallel over 8 NeuronCores (32 rows each). Rows are
globally sorted by length and dealt round-robin so all cores see the same
per-slot padded length; per-slot lengths are compile-time constants, which
makes the whole program static while only paying ~3% padded work.

Per slot (sample) on a core:
  - DMA x[j, :L] natural (pos-major) into SBUF
  - PE-transpose into xT (feat on partitions, positions free), via identity
  - mm1: 4 psum tiles (128 g x TL), each = 4 accumulated matmuls with the
    stationary W1 128x128 chunks; the "x[t+1]" half of the pair-concat is a
    free +1 column shift of xT
  - relu(+b1) PSUM->SBUF (alternating ACT/DVE)
  - mm2: scores (1 x TL) = 4 accumulated matmuls with w2 chunks stationary
  - copy scores into a (32, 512) batched scores matrix
Then one batched masked-softmax block computes logp-at-action and entropy
for all 32 rows and DMAs out (32, 2).
"""

import numpy as np
from contextlib import ExitStack

import concourse.bass as bass
import concourse.tile as tile
from concourse import mybir
from concourse.bass_utils import run_bass_kernel_spmd
import bass_rust

F32 = mybir.dt.float32
N_CORES = 8
B, S, E = 256, 512, 256
BC = B // N_CORES  # rows per core
G = 2 * E          # 512 hidden
NEG = -1e9

AF = mybir.ActivationFunctionType
ALU = mybir.AluOpType
AX = mybir.AxisListType

# --------------------------------------------------------------------------
# walrus in this toolchain rejects instructions with more than one sync wait
# ("Too many sync wait commands"); split extras onto preceding same-engine
# NOP carriers.
_MAXW = 1


def _split_sync_waits(nc):
    for bb in nc.main_func.blocks:
        il = bb.instructions
        i = 0
        while i < len(il):
            ins = il[i]
            si = ins.sync_info
            if si is not None and len(si.on_wait) > _MAXW:
                waits = list(si.on_wait)
                keep, rest = waits[-_MAXW:], waits[:-_MAXW]
                ins.sync_info = bass_rust.SyncInfo(
                    on_wait=keep, on_update=list(si.on_update))
                carriers = []
                for k in range(0, len(rest), _MAXW):
                    nop = mybir.InstNoOp(
                        name=f"waitsplit-{nc.next_id()}", ins=[], outs=[])
                    nop.engine = ins.engine
                    nop.sync_info = bass_rust.SyncInfo(
                        on_wait=rest[k:k + _MAXW], on_update=[])
                    carriers.append(nop)
                for j, nop in enumerate(carriers):
                    il.insert(i + j, nop)
                i += len(carriers)
            i += 1


# --------------------------------------------------------------------------
def _build_program(slot_len):
    """Emit the SPMD program. slot_len: 32 compile-time padded lengths."""
    nc = bass.Bass()

    x_d = nc.declare_dram_parameter("x", [BC, S, E], F32, isOutput=False)
    w1_d = nc.declare_dram_parameter("w1", [128, 16, 128], F32, isOutput=False)
    w2_d = nc.declare_dram_parameter("w2", [128, 4], F32, isOutput=False)
    b1_d = nc.declare_dram_parameter("b1", [128, 4], F32, isOutput=False)
    mb_d = nc.declare_dram_parameter("maskbias", [BC, S], F32, isOutput=False)
    oh_d = nc.declare_dram_parameter("onehot", [BC, S], F32, isOutput=False)
    id_d = nc.declare_dram_parameter("ident", [128, 128], F32, isOutput=False)
    out_d = nc.declare_dram_parameter("out", [BC, 2], F32, isOutput=True)

    with ExitStack() as ctx:
        tc = ctx.enter_context(tile.TileContext(nc))
        singles = ctx.enter_context(tc.tile_pool(name="singles", bufs=1))
        xnat_p = ctx.enter_context(tc.tile_pool(name="xnat", bufs=3))
        xtps_p = ctx.enter_context(tc.tile_pool(name="xtps", bufs=2, space="PSUM"))
        xt_p = ctx.enter_context(tc.tile_pool(name="xt", bufs=4))
        hps_p = ctx.enter_context(tc.tile_pool(name="hps", bufs=4, space="PSUM"))
        h_p = ctx.enter_context(tc.tile_pool(name="h", bufs=8))
        scps_p = ctx.enter_context(tc.tile_pool(name="scps", bufs=2, space="PSUM"))
        stage_p = ctx.enter_context(tc.tile_pool(name="stage", bufs=4))
        sm_p = ctx.enter_context(tc.tile_pool(name="sm", bufs=1))

        # --- one-time loads -------------------------------------------------
        w1_sb = singles.tile([128, 16, 128], F32)
        nc.sync.dma_start(out=w1_sb, in_=w1_d[:, :, :])
        w2_sb = singles.tile([128, 4], F32)
        nc.sync.dma_start(out=w2_sb, in_=w2_d[:, :])
        b1_sb = singles.tile([128, 4], F32)
        nc.sync.dma_start(out=b1_sb, in_=b1_d[:, :])
        id_sb = singles.tile([128, 128], F32)
        nc.sync.dma_start(out=id_sb, in_=id_d[:, :])
        mb_sb = singles.tile([BC, S], F32)
        nc.sync.dma_start(out=mb_sb, in_=mb_d[:, :])
        oh_sb = singles.tile([BC, S], F32)
        nc.sync.dma_start(out=oh_sb, in_=oh_d[:, :])

        scores_all = singles.tile([BC, S], F32)
        nc.vector.memset(scores_all, 0.0)

        # Pull the exp/ln activation tables in early so the ~2.7us table DMA
        # overlaps the main pipeline instead of landing in the tail.
        warm = singles.tile([1, 2], F32)
        nc.vector.memset(warm, 1.0)
        nc.scalar.activation(warm[:, 0:1], warm[:, 0:1], AF.Exp)
        nc.scalar.activation(warm[:, 1:2], warm[:, 1:2], AF.Ln)

        # --- per-slot pipeline ----------------------------------------------
        for j in range(BC):
            L = int(slot_len[j])
            TL = L - 1                      # valid score positions
            nfull, rem = divmod(L, 128)
            C = nfull + (1 if rem else 0)   # 128-position chunks

            xnat = xnat_p.tile([128, 4, 256], F32, tag="xnat")
            if nfull:
                nc.sync.dma_start(
                    out=xnat[:, 0:nfull, :],
                    in_=x_d[j, 0:nfull * 128, :].rearrange(
                        "(c p) e -> p c e", p=128))
            if rem:
                nc.sync.dma_start(
                    out=xnat[0:rem, nfull, :],
                    in_=x_d[j, nfull * 128:L, :])

            # transpose x -> xT (two 128-feature halves, positions on free)
            xts = []
            for e2 in range(2):
                xt_ps = xtps_p.tile([128, 512], F32, tag="xtps")
                for c in range(C):
                    pc = min(128, L - 128 * c)
                    nc.tensor.transpose(
                        out=xt_ps[:, 128 * c:128 * c + pc],
                        in_=xnat[0:pc, c, 128 * e2:128 * (e2 + 1)],
                        identity=id_sb[0:pc, 0:pc])
                xt = xt_p.tile([128, 512], F32, tag="xt")
                if e2 == 0:
                    nc.scalar.copy(out=xt[:, 0:L], in_=xt_ps[:, 0:L])
                else:
                    nc.vector.tensor_copy(out=xt[:, 0:L], in_=xt_ps[:, 0:L])
                xts.append(xt)

            # mm1 + relu per 128-wide g block; then mm2 accumulation
            sc_ps = scps_p.tile([1, 512], F32, tag="scps")
            for g in range(4):
                hp = hps_p.tile([128, 512], F32, tag="hps")
                # chunk index c = e*4+g, e in 0..3 (0,1: W1 top; 2,3: bottom)
                nc.tensor.matmul(hp[:, 0:TL], w1_sb[:, 0 * 4 + g, :],
                                 xts[0][:, 0:TL], start=True, stop=False)
                nc.tensor.matmul(hp[:, 0:TL], w1_sb[:, 1 * 4 + g, :],
                                 xts[1][:, 0:TL], start=False, stop=False)
                nc.tensor.matmul(hp[:, 0:TL], w1_sb[:, 2 * 4 + g, :],
                                 xts[0][:, 1:L], start=False, stop=False)
                nc.tensor.matmul(hp[:, 0:TL], w1_sb[:, 3 * 4 + g, :],
                                 xts[1][:, 1:L], start=False, stop=True)

                h = h_p.tile([128, 512], F32, tag="h")
                if g % 2 == 0:
                    nc.scalar.activation(h[:, 0:TL], hp[:, 0:TL], AF.Relu,
                                         bias=b1_sb[:, g:g + 1], scale=1.0)
                else:
                    nc.vector.tensor_scalar(h[:, 0:TL], hp[:, 0:TL],
                                            b1_sb[:, g:g + 1], 0.0,
                                            op0=ALU.add, op1=ALU.max)
                nc.tensor.matmul(sc_ps[0:1, 0:TL], w2_sb[:, g:g + 1],
                                 h[:, 0:TL], start=(g == 0), stop=(g == 3))

            # engines cannot write a single non-32-aligned partition, so
            # stage on partition 0 and scatter with a SBUF->SBUF DMA
            stg = stage_p.tile([1, 512], F32, tag="stage")
            if j % 2 == 0:
                nc.scalar.copy(out=stg[0:1, 0:TL], in_=sc_ps[0:1, 0:TL])
            else:
                nc.vector.tensor_copy(out=stg[0:1, 0:TL], in_=sc_ps[0:1, 0:TL])
            nc.sync.dma_start(out=scores_all[j:j + 1, 0:TL],
                              in_=stg[0:1, 0:TL])

        # --- batched masked softmax / entropy --------------------------------
        logits = sm_p.tile([BC, S], F32)
        nc.vector.tensor_add(logits, scores_all, mb_sb)
        rowmax = sm_p.tile([BC, 1], F32)
        nc.vector.reduce_max(rowmax, logits, axis=AX.X)
        zt = sm_p.tile([BC, S], F32)
        nc.vector.tensor_scalar_sub(zt, logits, rowmax)
        et = sm_p.tile([BC, S], F32)
        sumexp = sm_p.tile([BC, 1], F32)
        nc.scalar.activation(et, zt, AF.Exp, accum_out=sumexp)
        logsum = sm_p.tile([BC, 1], F32)
        nc.scalar.activation(logsum, sumexp, AF.Ln)
        rinv = sm_p.tile([BC, 1], F32)
        nc.vector.reciprocal(rinv, sumexp)
        logp = sm_p.tile([BC, S], F32)
        nc.vector.tensor_scalar_sub(logp, zt, logsum)

        scr0 = sm_p.tile([BC, S], F32)
        lp = sm_p.tile([BC, 1], F32)
        nc.vector.tensor_mul(scr0, logp, oh_sb)
        nc.vector.reduce_sum(lp, scr0, axis=AX.X)
        scr1 = sm_p.tile([BC, S], F32)
        ez = sm_p.tile([BC, 1], F32)
        nc.vector.tensor_mul(scr1, et, zt)
        nc.vector.reduce_sum(ez, scr1, axis=AX.X)
        # entropy = logsum - (sum e*z) / sumexp
        ent = sm_p.tile([BC, 1], F32)
        nc.vector.tensor_mul(ent, ez, rinv)
        nc.vector.tensor_sub(ent, logsum, ent)

        res = sm_p.tile([BC, 2], F32)
        nc.vector.tensor_copy(res[:, 0:1], lp)
        nc.vector.tensor_copy(res[:, 1:2], ent)
        nc.sync.dma_start(out=out_d[:, :], in_=res)

    _split_sync_waits(nc)
    return nc


# --------------------------------------------------------------------------
_prog_cache = {}
LAST_RESULT = None


def kernel(x, W1, b1, W2, b2, lengths, position_action):
    x = np.ascontiguousarray(np.asarray(x, np.float32))
    W1 = np.asarray(W1, np.float32)
    b1 = np.asarray(b1, np.float32)
    W2 = np.asarray(W2, np.float32)
    b2 = np.asarray(b2, np.float32)
    lengths = np.asarray(lengths)
    position_action = np.asarray(position_action)

    # length-sorted round-robin assignment: rank r -> core r%8, slot r//8
    order = np.argsort(lengths, kind="stable")
    slot_len = [int(lengths[order[j * N_CORES + N_CORES - 1]])
                for j in range(BC)]

    key = tuple(slot_len)
    if key not in _prog_cache:
        _prog_cache[key] = _build_program(slot_len)
    nc = _prog_cache[key]

    # replicated params, pre-chunked for the 128x128 stationary loads
    w1c = np.ascontiguousarray(
        W1.reshape(4, 128, 4, 128).transpose(1, 0, 2, 3).reshape(128, 16, 128))
    w2c = np.ascontiguousarray(W2[:, 0].reshape(4, 128).T)
    b1c = np.ascontiguousarray(b1.reshape(4, 128).T)
    ident = np.eye(128, dtype=np.float32)

    tcol = np.arange(S, dtype=np.int64)[None, :]
    in_maps = []
    core_rows = []
    for core in range(N_CORES):
        rows = order[np.arange(BC) * N_CORES + core]
        core_rows.append(rows)
        lens = lengths[rows].astype(np.int64)
        mb = np.where(tcol < (lens - 1)[:, None], np.float32(0), np.float32(NEG))
        oh = np.zeros((BC, S), np.float32)
        oh[np.arange(BC), position_action[rows].astype(np.int64)] = 1.0
        in_maps.append({
            "x": np.ascontiguousarray(x[rows]),
            "w1": w1c, "w2": w2c, "b1": b1c,
            "maskbias": np.ascontiguousarray(mb.astype(np.float32)),
            "onehot": oh,
            "ident": ident,
        })

    br = run_bass_kernel_spmd(nc, in_maps, list(range(N_CORES)))
    global LAST_RESULT
    LAST_RESULT = br

    out = np.zeros((B, 2), np.float32)
    for core in range(N_CORES):
        out[core_rows[core]] = br.results[core]["out"]
    return out


# revision 16
# speedup vs baseline: 26250.1265x; 26250.1265x over previous
"""Trainium2 Bass kernel for nn_ActorCritic_38886633898257.

Computes, for each batch row b of x (B, S, E):
  pairs[t]  = concat(x[b, t], x[b, t+1])            t in [0, S-2]
  h         = relu(pairs @ W1 + b1)
  scores[t] = h @ W2[:, 0]                          (+ b2, shift-invariant)
  logits    = scores masked to t < len_b - 1
  logp      = log_softmax(logits)
  out[b]    = (logp[action_b], entropy(logits))

Strategy: pure data parallel over 8 NeuronCores (32 rows each). Rows are
globally sorted by length and dealt round-robin so all cores see the same
per-slot padded length; per-slot lengths are compile-time constants, which
makes the whole program static while only paying ~3% padded work.

Per slot (sample) on a core:
  - DMA x[j, :L] natural (pos-major) into SBUF
  - PE-transpose into xT (feat on partitions, positions free), via identity
  - mm1: 4 psum tiles (128 g x TL), each = 4 accumulated matmuls with the
    stationary W1 128x128 chunks; the "x[t+1]" half of the pair-concat is a
    free +1 column shift of xT
  - relu(+b1) PSUM->SBUF (alternating ACT/DVE)
  - mm2: scores (1 x TL) = 4 accumulated matmuls with w2 chunks stationary
  - copy scores into a (32, 512) batched scores matrix
Then one batched masked-softmax block computes logp-at-action and entropy
for all 32 rows and DMAs out (32, 2).
"""

import numpy as np
from contextlib import ExitStack

import concourse.bass as bass
import concourse.tile as tile
from concourse import mybir
from concourse.bass_utils import run_bass_kernel_spmd
import bass_rust

F32 = mybir.dt.float32
N_CORES = 8
B, S, E = 256, 512, 256
BC = B // N_CORES  # rows per core
G = 2 * E          # 512 hidden
NEG = -1e9

AF = mybir.ActivationFunctionType
ALU = mybir.AluOpType
AX = mybir.AxisListType

# --------------------------------------------------------------------------
# walrus in this toolchain rejects instructions with more than one sync wait
# ("Too many sync wait commands"); split extras onto preceding same-engine
# NOP carriers.
_MAXW = 1


def _split_sync_waits(nc):
    for bb in nc.main_func.blocks:
        il = bb.instructions
        i = 0
        while i < len(il):
            ins = il[i]
            si = ins.sync_info
            if si is not None and len(si.on_wait) > _MAXW:
                waits = list(si.on_wait)
                keep, rest = waits[-_MAXW:], waits[:-_MAXW]
                ins.sync_info = bass_rust.SyncInfo(
                    on_wait=keep, on_update=list(si.on_update))
                carriers = []
                for k in range(0, len(rest), _MAXW):
                    nop = mybir.InstNoOp(
                        name=f"waitsplit-{nc.next_id()}", ins=[], outs=[])
                    nop.engine = ins.engine
                    nop.sync_info = bass_rust.SyncInfo(
                        on_wait=rest[k:k + _MAXW], on_update=[])
                    carriers.append(nop)
                for j, nop in enumerate(carriers):
                    il.insert(i + j, nop)
                i += len(carriers)
            i += 1


# --------------------------------------------------------------------------
def _build_program(slot_len, repeat=1):
    """Emit the SPMD program. slot_len: 32 compile-time padded lengths.

    repeat > 1 replicates the whole compute pipeline (timing amplification
    only; the final write still produces correct results)."""
    nc = bass.Bass()

    x_d = nc.declare_dram_parameter("x", [BC, S, E], F32, isOutput=False)
    w1_d = nc.declare_dram_parameter("w1", [128, 16, 128], F32, isOutput=False)
    w2_d = nc.declare_dram_parameter("w2", [128, 4], F32, isOutput=False)
    b1_d = nc.declare_dram_parameter("b1", [128, 4], F32, isOutput=False)
    mb_d = nc.declare_dram_parameter("maskbias", [BC, S], F32, isOutput=False)
    oh_d = nc.declare_dram_parameter("onehot", [BC, S], F32, isOutput=False)
    id_d = nc.declare_dram_parameter("ident", [128, 128], F32, isOutput=False)
    out_d = nc.declare_dram_parameter("out", [BC, 2], F32, isOutput=True)

    with ExitStack() as ctx:
        tc = ctx.enter_context(tile.TileContext(nc))
        singles = ctx.enter_context(tc.tile_pool(name="singles", bufs=1))
        xnat_p = ctx.enter_context(tc.tile_pool(name="xnat", bufs=3))
        xtps_p = ctx.enter_context(tc.tile_pool(name="xtps", bufs=2, space="PSUM"))
        xt_p = ctx.enter_context(tc.tile_pool(name="xt", bufs=4))
        hps_p = ctx.enter_context(tc.tile_pool(name="hps", bufs=4, space="PSUM"))
        h_p = ctx.enter_context(tc.tile_pool(name="h", bufs=8))
        scps_p = ctx.enter_context(tc.tile_pool(name="scps", bufs=2, space="PSUM"))
        stage_p = ctx.enter_context(tc.tile_pool(name="stage", bufs=4))
        sm_p = ctx.enter_context(tc.tile_pool(name="sm", bufs=1))

        # --- one-time loads -------------------------------------------------
        w1_sb = singles.tile([128, 16, 128], F32)
        nc.sync.dma_start(out=w1_sb, in_=w1_d[:, :, :])
        w2_sb = singles.tile([128, 4], F32)
        nc.sync.dma_start(out=w2_sb, in_=w2_d[:, :])
        b1_sb = singles.tile([128, 4], F32)
        nc.sync.dma_start(out=b1_sb, in_=b1_d[:, :])
        id_sb = singles.tile([128, 128], F32)
        nc.sync.dma_start(out=id_sb, in_=id_d[:, :])
        mb_sb = singles.tile([BC, S], F32)
        nc.sync.dma_start(out=mb_sb, in_=mb_d[:, :])
        oh_sb = singles.tile([BC, S], F32)
        nc.sync.dma_start(out=oh_sb, in_=oh_d[:, :])

        scores_all = singles.tile([BC, S], F32)
        nc.vector.memset(scores_all, 0.0)

        # Pull the exp/ln activation tables in early so the ~2.7us table DMA
        # overlaps the main pipeline instead of landing in the tail.
        warm = singles.tile([1, 2], F32)
        nc.vector.memset(warm, 1.0)
        nc.scalar.activation(warm[:, 0:1], warm[:, 0:1], AF.Exp)
        nc.scalar.activation(warm[:, 1:2], warm[:, 1:2], AF.Ln)

        # --- per-slot pipeline ----------------------------------------------
        for rep in range(repeat):
          scores_all = scores_all if rep == 0 else singles.tile([BC, S], F32)
          if rep > 0:
              nc.vector.memset(scores_all, 0.0)
          for j in range(BC):
            L = int(slot_len[j])
            TL = L - 1                      # valid score positions
            nfull, rem = divmod(L, 128)
            C = nfull + (1 if rem else 0)   # 128-position chunks

            xnat = xnat_p.tile([128, 4, 256], F32, tag="xnat")
            if nfull:
                nc.sync.dma_start(
                    out=xnat[:, 0:nfull, :],
                    in_=x_d[j, 0:nfull * 128, :].rearrange(
                        "(c p) e -> p c e", p=128))
            if rem:
                nc.sync.dma_start(
                    out=xnat[0:rem, nfull, :],
                    in_=x_d[j, nfull * 128:L, :])

            # transpose x -> xT (two 128-feature halves, positions on free)
            xts = []
            for e2 in range(2):
                xt_ps = xtps_p.tile([128, 512], F32, tag="xtps")
                for c in range(C):
                    pc = min(128, L - 128 * c)
                    nc.tensor.transpose(
                        out=xt_ps[:, 128 * c:128 * c + pc],
                        in_=xnat[0:pc, c, 128 * e2:128 * (e2 + 1)],
                        identity=id_sb[0:pc, 0:pc])
                xt = xt_p.tile([128, 512], F32, tag="xt")
                if e2 == 0:
                    nc.scalar.copy(out=xt[:, 0:L], in_=xt_ps[:, 0:L])
                else:
                    nc.vector.tensor_copy(out=xt[:, 0:L], in_=xt_ps[:, 0:L])
                xts.append(xt)

            # mm1 + relu per 128-wide g block; then mm2 accumulation
            sc_ps = scps_p.tile([1, 512], F32, tag="scps")
            for g in range(4):
                hp = hps_p.tile([128, 512], F32, tag="hps")
                # chunk index c = e*4+g, e in 0..3 (0,1: W1 top; 2,3: bottom)
                nc.tensor.matmul(hp[:, 0:TL], w1_sb[:, 0 * 4 + g, :],
                                 xts[0][:, 0:TL], start=True, stop=False)
                nc.tensor.matmul(hp[:, 0:TL], w1_sb[:, 1 * 4 + g, :],
                                 xts[1][:, 0:TL], start=False, stop=False)
                nc.tensor.matmul(hp[:, 0:TL], w1_sb[:, 2 * 4 + g, :],
                                 xts[0][:, 1:L], start=False, stop=False)
                nc.tensor.matmul(hp[:, 0:TL], w1_sb[:, 3 * 4 + g, :],
                                 xts[1][:, 1:L], start=False, stop=True)

                h = h_p.tile([128, 512], F32, tag="h")
                if g % 2 == 0:
                    nc.scalar.activation(h[:, 0:TL], hp[:, 0:TL], AF.Relu,
                                         bias=b1_sb[:, g:g + 1], scale=1.0)
                else:
                    nc.vector.tensor_scalar(h[:, 0:TL], hp[:, 0:TL],
                                            b1_sb[:, g:g + 1], 0.0,
                                            op0=ALU.add, op1=ALU.max)
                nc.tensor.matmul(sc_ps[0:1, 0:TL], w2_sb[:, g:g + 1],
                                 h[:, 0:TL], start=(g == 0), stop=(g == 3))

            # engines cannot write a single non-32-aligned partition, so
            # stage on partition 0 and scatter with a SBUF->SBUF DMA
            stg = stage_p.tile([1, 512], F32, tag="stage")
            if j % 2 == 0:
                nc.scalar.copy(out=stg[0:1, 0:TL], in_=sc_ps[0:1, 0:TL])
            else:
                nc.vector.tensor_copy(out=stg[0:1, 0:TL], in_=sc_ps[0:1, 0:TL])
            nc.sync.dma_start(out=scores_all[j:j + 1, 0:TL],
                              in_=stg[0:1, 0:TL])

        # --- batched masked softmax / entropy --------------------------------
        logits = sm_p.tile([BC, S], F32)
        nc.vector.tensor_add(logits, scores_all, mb_sb)
        rowmax = sm_p.tile([BC, 1], F32)
        nc.vector.reduce_max(rowmax, logits, axis=AX.X)
        zt = sm_p.tile([BC, S], F32)
        nc.vector.tensor_scalar_sub(zt, logits, rowmax)
        et = sm_p.tile([BC, S], F32)
        sumexp = sm_p.tile([BC, 1], F32)
        nc.scalar.activation(et, zt, AF.Exp, accum_out=sumexp)
        logsum = sm_p.tile([BC, 1], F32)
        nc.scalar.activation(logsum, sumexp, AF.Ln)
        rinv = sm_p.tile([BC, 1], F32)
        nc.vector.reciprocal(rinv, sumexp)
        logp = sm_p.tile([BC, S], F32)
        nc.vector.tensor_scalar_sub(logp, zt, logsum)

        scr0 = sm_p.tile([BC, S], F32)
        lp = sm_p.tile([BC, 1], F32)
        nc.vector.tensor_mul(scr0, logp, oh_sb)
        nc.vector.reduce_sum(lp, scr0, axis=AX.X)
        scr1 = sm_p.tile([BC, S], F32)
        ez = sm_p.tile([BC, 1], F32)
        nc.vector.tensor_mul(scr1, et, zt)
        nc.vector.reduce_sum(ez, scr1, axis=AX.X)
        # entropy = logsum - (sum e*z) / sumexp
        ent = sm_p.tile([BC, 1], F32)
        nc.vector.tensor_mul(ent, ez, rinv)
        nc.vector.tensor_sub(ent, logsum, ent)

        res = sm_p.tile([BC, 2], F32)
        nc.vector.tensor_copy(res[:, 0:1], lp)
        nc.vector.tensor_copy(res[:, 1:2], ent)
        nc.sync.dma_start(out=out_d[:, :], in_=res)

    _split_sync_waits(nc)
    return nc


# --------------------------------------------------------------------------
_prog_cache = {}
LAST_RESULT = None


def kernel(x, W1, b1, W2, b2, lengths, position_action):
    x = np.ascontiguousarray(np.asarray(x, np.float32))
    W1 = np.asarray(W1, np.float32)
    b1 = np.asarray(b1, np.float32)
    W2 = np.asarray(W2, np.float32)
    b2 = np.asarray(b2, np.float32)
    lengths = np.asarray(lengths)
    position_action = np.asarray(position_action)

    # length-sorted round-robin assignment: rank r -> core r%8, slot r//8
    order = np.argsort(lengths, kind="stable")
    slot_len = [int(lengths[order[j * N_CORES + N_CORES - 1]])
                for j in range(BC)]

    key = tuple(slot_len)
    if key not in _prog_cache:
        _prog_cache[key] = _build_program(slot_len)
    nc = _prog_cache[key]

    # replicated params, pre-chunked for the 128x128 stationary loads
    w1c = np.ascontiguousarray(
        W1.reshape(4, 128, 4, 128).transpose(1, 0, 2, 3).reshape(128, 16, 128))
    w2c = np.ascontiguousarray(W2[:, 0].reshape(4, 128).T)
    b1c = np.ascontiguousarray(b1.reshape(4, 128).T)
    ident = np.eye(128, dtype=np.float32)

    tcol = np.arange(S, dtype=np.int64)[None, :]
    in_maps = []
    core_rows = []
    for core in range(N_CORES):
        rows = order[np.arange(BC) * N_CORES + core]
        core_rows.append(rows)
        lens = lengths[rows].astype(np.int64)
        mb = np.where(tcol < (lens - 1)[:, None], np.float32(0), np.float32(NEG))
        oh = np.zeros((BC, S), np.float32)
        oh[np.arange(BC), position_action[rows].astype(np.int64)] = 1.0
        in_maps.append({
            "x": np.ascontiguousarray(x[rows]),
            "w1": w1c, "w2": w2c, "b1": b1c,
            "maskbias": np.ascontiguousarray(mb.astype(np.float32)),
            "onehot": oh,
            "ident": ident,
        })

    br = run_bass_kernel_spmd(nc, in_maps, list(range(N_CORES)))
    global LAST_RESULT
    LAST_RESULT = br

    out = np.zeros((B, 2), np.float32)
    for core in range(N_CORES):
        out[core_rows[core]] = br.results[core]["out"]
    return out
